# revision 76
# baseline (speedup 1.0000x reference)
"""Trainium2 Bass kernel for nn_Attention_40372692582854.

Single-head attention block: LayerNorm -> QKV -> softmax(QK^T*sc)@V -> out
projection -> gelu(out + x).  Data-parallel over batch: 8 batch elements,
one per NeuronCore.

All matmuls run as fp8 (e4m3) DoubleRow: a dual-row fp8 matmul contracts
K=256 per pass at the same per-instruction cost as a bf16 K=128 matmul
(measured ~220-235ns per 512-col matmul either way), halving the PE
instruction stream.  Paired operands hold two consecutive 128-row K-slices
in a middle dim of 2 (16B-aligned strides per the dual-fp8 ISA rules).
Measured rel err vs the fp32 reference: 1.7e-2 (gate 2e-2); numerics were
validated operand-by-operand against a numpy fp8 model.

Per-core dataflow (S=2048 tokens, D=768 dims):
  - Phase A, per 512-token group: LN stats via bn_stats; x1c=(x-mu)*rstd in
    bf16 (gamma/beta folded host-side), PE-transposed 128x128 pair-wise into
    fp8 x1cT [128,2,512] tiles.  Then in the same group:
      v[s,dv]   = x1cT.T @ wv   (3 dual mm per h-chunk, fp8 out, bias
                                 folded into bo on host: bo' = bo + bv@wo)
      kT/qT[dk,s] = wqk.T @ x1cT + b  (3 dual mm, bias in the fp8 cast,
                                 unscaled: 1/sqrt(D) is applied in the exp)
    x stays resident in SBUF for the phase-C residual.
  - Phase B, attention per 512-query chunk (all fp8 DoubleRow):
      scoresT[k,q] = kT.T @ qT          (3 dual mm)
      pT = exp(sc*scoresT - ln 16)      (ACT, fp8 out)
      den[1,q] via ones dual mm; outT[dv,q]/16 = v.T @ pT -> fp8 pairs.
    Chunk c-1's output matmuls are interleaved into chunk c's score stream
    so PE never waits on ACT's exp and keeps its p-state high.
  - Phase C: y = gelu((outT.T @ wo) * (16/den) + bo' + x), two token tiles
    per iteration (wide DVE/Pool/ACT ops, single in/out DMAs).
"""

import math
import numpy as np
import ml_dtypes
from contextlib import ExitStack

import concourse.bass as bass
import concourse.tile as tile
import concourse.mybir as mybir
from concourse import bacc
from concourse.masks import make_identity
from concourse.bass_utils import run_bass_kernel_spmd

F32 = mybir.dt.float32
BF16 = mybir.dt.bfloat16
F8 = mybir.dt.float8e4
AF = mybir.ActivationFunctionType
OP = mybir.AluOpType
DR = mybir.MatmulPerfMode.DoubleRow

B = 8
S = 2048
D = 768
P = 128
DT = D // P            # 6 dim tiles
DP = DT // 2           # 3 dim pairs
ST = S // P            # 16 token tiles
SC = 512               # matmul moving free dim
NSC = S // SC          # 4 token chunks
EPS = 1e-5
SCALE = D ** -0.5
PDIV = 16.0            # p = exp(sc*s)/PDIV so fp8 sees well-scaled values


def ts(i, n):
    return bass.ts(i, n)


def build_bass(reps=1, phases="ABC"):
    if "A" in phases:
        phases = phases.replace("A", "LVK")
    nc = bacc.Bacc("TRN2")

    x_d = nc.dram_tensor("x", [S, D], F32, kind="ExternalInput")
    wqk_d = nc.dram_tensor("wqk", [DP, P, 2, 2 * D], F8, kind="ExternalInput")
    wv_d = nc.dram_tensor("wv", [DP, P, 2, D], F8, kind="ExternalInput")
    wo_d = nc.dram_tensor("wo", [DP, P, 2, D], F8, kind="ExternalInput")
    bqk_d = nc.dram_tensor("bqk", [P, 2 * DT], F32, kind="ExternalInput")
    bo_d = nc.dram_tensor("bo", [P, D], F32, kind="ExternalInput")
    out_d = nc.dram_tensor("out", [S, D], F32, kind="ExternalOutput")

    with tile.TileContext(nc) as tc, \
         nc.allow_low_precision(reason="fp8 kernel, validated vs reference"):
      for _rep in range(reps):
        with ExitStack() as ctx:
          const = ctx.enter_context(tc.tile_pool(name="const", bufs=1))
          big = ctx.enter_context(tc.tile_pool(name="big", bufs=1))

          # ---- long-lived constants ----
          wo_t = [const.tile([P, 2, D], F8, tag=f"wo{i}", name=f"wo{i}")
                  for i in range(DP)]
          bo_t = const.tile([P, D], F32, tag="bo", name="bo")
          # [P, 2, 16] so the dual-fp8 ldweights outer free stride is 16B
          ones_t = const.tile([P, 2, 16], F8, tag="ones", name="ones")
          nc.vector.memset(ones_t, 1.0)
          ident = const.tile([P, P], BF16, tag="ident", name="ident")
          make_identity(nc, ident)
          nexp_b = const.tile([P, 1], F32, tag="nexp_b", name="nexp_b")
          nc.vector.memset(nexp_b, -math.log(PDIV))

          # ---- persistent activations (fp8 pairs for DoubleRow) ----
          v_t = [big.tile([P, 2, D], F8, tag=f"v{t2}", name=f"v{t2}")
                 for t2 in range(ST // 2)]
          kT = [big.tile([P, 2, S], F8, tag=f"kT{j}", name=f"kT{j}")
                for j in range(DP)]
          qT = [big.tile([P, 2, S], F8, tag=f"qT{j}", name=f"qT{j}")
                for j in range(DP)]
          inv_den = big.tile([P, ST], F32, tag="inv_den", name="inv_den")
          outT = [big.tile([P, 2, S], F8, tag=f"outT{op}", name=f"outT{op}")
                  for op in range(DP)]
          mvall = big.tile([P, 2 * ST], F32, tag="mvall", name="mvall")
          invall = big.tile([P, ST], F32, tag="invall", name="invall")
          # x stays resident for the phase-C residual (saves a 6MB reload)
          x_res = [big.tile([P, D], F32, tag=f"x{t}", name=f"x{t}")
                   for t in range(ST)]

          # ========= Phase A: LN + transpose + V/K/Q, per token group =======
          if "L" in phases:
           with tc.tile_pool(name="wpool", bufs=1) as wp, \
               tc.tile_pool(name="ln", bufs=6) as ln, \
               tc.tile_pool(name="proj", bufs=2, space="PSUM") as proj, \
               tc.tile_pool(name="xt", bufs=2) as xtp:
              wqk_t = [wp.tile([P, 2, 2 * D], F8, tag=f"wqk{i}", name=f"wqk{i}")
                       for i in range(DP)]
              wv_t = [wp.tile([P, 2, D], F8, tag=f"wv{i}", name=f"wv{i}")
                      for i in range(DP)]
              bqk_t = wp.tile([P, 2 * DT], F32, tag="bqk", name="bqk")
              # weights go on the gpsimd SWDGE queue so the x loads (sync
              # HWDGE) aren't queued behind the weight traffic
              for i in range(DP):
                  nc.gpsimd.dma_start(out=wv_t[i], in_=wv_d[i])
              for i in range(DP):
                  nc.gpsimd.dma_start(out=wqk_t[i], in_=wqk_d[i])
              nc.gpsimd.dma_start(out=bqk_t, in_=bqk_d[:, :])
              for i in range(DP):
                  nc.gpsimd.dma_start(out=wo_t[i], in_=wo_d[i])
              nc.gpsimd.dma_start(out=bo_t, in_=bo_d[:, :])
              eps_t = wp.tile([P, 1], F32, tag="eps", name="eps")
              nc.vector.memset(eps_t, EPS)

              for c in range(NSC):
                  # one tile [P, 2(pair), 3(jp), SC]: dual-pair dim stride
                  # 3*SC=1536B stays 16B-aligned for the fp8 ldweights
                  x1cT = xtp.tile([P, 2, DP, SC], F8, tag="x1cT",
                                  name="x1cT", bufs=2)
                  xts = []
                  for t in range(4 * c, 4 * c + 4):
                      x_t = x_res[t]
                      xts.append(x_t)
                      nc.sync.dma_start(out=x_t, in_=x_d[ts(t, P), :])
                      stats = ln.tile([P, 3, 6], F32, tag="stats", name="stats")
                      for sg in range(3):
                          nc.vector.bn_stats(out=stats[:, sg, :],
                                             in_=x_t[:, ts(sg, 256)])
                      nc.vector.bn_aggr(out=mvall[:, 2 * t:2 * t + 2], in_=stats)
                  # one batched sqrt over the 4 variances (strided AP)
                  stdb = ln.tile([P, 4], F32, tag="stdb", name="stdb")
                  nc.scalar.activation(
                      out=stdb,
                      in_=mvall[:, 8 * c: 8 * c + 8].rearrange(
                          "p (t two) -> p t two", two=2)[:, :, 1],
                      func=AF.Sqrt, bias=eps_t, scale=1.0)
                  nc.vector.reciprocal(out=invall[:, 4 * c:4 * c + 4], in_=stdb)
                  for tt, t in enumerate(range(4 * c, 4 * c + 4)):
                      x1c = ln.tile([P, D], BF16, tag="x1c", name="x1c", bufs=8)
                      nc.vector.tensor_scalar(out=x1c, in0=xts[tt],
                                              scalar1=mvall[:, 2 * t:2 * t + 1],
                                              scalar2=invall[:, t:t + 1],
                                              op0=OP.subtract, op1=OP.mult)
                      # all 6 transposes of the tile into one PSUM tile,
                      # blocks ordered (pair i, jp) so a single copy's AP
                      # iteration matches the x1cT [i, jp, col] layout
                      pst = proj.tile([P, DT * P], BF16, tag="ptr",
                                      name="pst", bufs=3)
                      for i in range(2):
                          for jp in range(DP):
                              nc.tensor.transpose(
                                  pst[:, ts(i * DP + jp, P)],
                                  x1c[:, ts(2 * jp + i, P)], ident)
                      if tt % 2 == 0:
                          nc.vector.tensor_copy(
                              out=x1cT[:, :, :, ts(tt, P)], in_=pst)
                      else:
                          nc.scalar.copy(
                              out=x1cT[:, :, :, ts(tt, P)], in_=pst)

                  # ---- V = x1 @ Wv for these 4 tiles (bias folded in bo) ----
                  for tt, t in enumerate(range(4 * c, 4 * c + 4) if "V" in phases else ()):
                      ps = proj.tile([P, D], F32, tag="mm", name="pv")
                      for h0, hn in ((0, 512), (512, 256)):
                          for jp in range(DP):
                              nc.tensor.matmul(
                                  ps[:, h0:h0 + hn],
                                  lhsT=x1cT[:, :, jp, ts(tt, P)],
                                  rhs=wv_t[jp][:, :, h0:h0 + hn],
                                  start=(jp == 0), stop=(jp == DP - 1),
                                  perf_mode=DR)
                      nc.scalar.copy(out=v_t[t // 2][:, t % 2, :], in_=ps)

                  # ---- kT, qT = W.T @ x1cT + bias for this token chunk ----
                  for which, dst in (((1, kT), (0, qT)) if "K" in phases else ()):
                      for j in range(DT):
                          bcol = bqk_t[:, which * DT + j: which * DT + j + 1]
                          pss = proj.tile([P, SC], F32, tag="mm", name="pkq",
                                          padded_shape=[P, D])
                          for jp in range(DP):
                              nc.tensor.matmul(
                                  pss,
                                  lhsT=wqk_t[jp][:, :, which * D + j * P:
                                                 which * D + (j + 1) * P],
                                  rhs=x1cT[:, :, jp, :],
                                  start=(jp == 0), stop=(jp == DP - 1),
                                  perf_mode=DR)
                          nc.scalar.activation(
                              out=dst[j // 2][:, j % 2, ts(c, SC)], in_=pss,
                              func=AF.Identity, bias=bcol, scale=1.0)

          # ====== Phase B: attention per q-chunk, fp8 DoubleRow, pipelined ==
          if "B" in phases:
           with tc.tile_pool(name="att", bufs=2) as att, \
               tc.tile_pool(name="att2", bufs=2) as att2, \
               tc.tile_pool(name="dram", bufs=2, space="DRAM") as dram, \
               tc.tile_pool(name="ps_s", bufs=4, space="PSUM") as pssp, \
               tc.tile_pool(name="ps_o", bufs=2, space="PSUM") as posp, \
               tc.tile_pool(name="pden", bufs=2, space="PSUM") as pdenp:

              def issue_scores(c, kt, pT):
                  ps_s = pssp.tile([P, SC], F32, tag="ps_s", name="ps_s",
                                   bufs=4)
                  for jj in range(DP):
                      nc.tensor.matmul(ps_s,
                                       lhsT=kT[jj][:, :, ts(kt, P)],
                                       rhs=qT[jj][:, :, ts(c, SC)],
                                       start=(jj == 0), stop=(jj == DP - 1),
                                       perf_mode=DR)
                  # pT = exp(scale*s - ln PDIV), cast to fp8
                  nc.scalar.activation(out=pT[kt // 2][:, kt % 2, :], in_=ps_s,
                                       func=AF.Exp, bias=nexp_b, scale=SCALE)

              def issue_out(c, ot, pT):
                  ps_o = posp.tile([P, SC], F32, tag="ps_o", name="ps_o",
                                   bufs=2)
                  for k2 in range(ST // 2):
                      nc.tensor.matmul(ps_o,
                                       lhsT=v_t[k2][:, :, ts(ot, P)],
                                       rhs=pT[k2],
                                       start=(k2 == 0), stop=(k2 == ST // 2 - 1),
                                       perf_mode=DR)
                  nc.vector.tensor_copy(out=outT[ot // 2][:, ot % 2, ts(c, SC)],
                                        in_=ps_o)

              def issue_den(c, pT):
                  ps_den = pdenp.tile([1, SC], F32, tag="pden", name="ps_den",
                                      bufs=2)
                  for k2 in range(ST // 2):
                      nc.tensor.matmul(ps_den, lhsT=ones_t[:, :, 0:1],
                                       rhs=pT[k2],
                                       start=(k2 == 0), stop=(k2 == ST // 2 - 1),
                                       perf_mode=DR)
                  # denominator -> per-partition reciprocal via DRAM bounce
                  den_row = att2.tile([1, SC], F32, tag="den_row",
                                      name="den_row")
                  nc.vector.tensor_copy(out=den_row, in_=ps_den)
                  den_b = dram.tile([1, SC], F32, tag="den_b", name="den_b")
                  nc.sync.dma_start(out=den_b, in_=den_row)
                  den_pp = att2.tile([P, NSC], F32, tag="den_pp", name="den_pp")
                  nc.sync.dma_start(out=den_pp,
                                    in_=den_b.rearrange("a (t p) -> (a p) t",
                                                        p=P))
                  nc.vector.reciprocal(out=inv_den[:, c * NSC:(c + 1) * NSC],
                                       in_=den_pp)

              prev = None
              for c in range(NSC):
                  pT = [att.tile([P, 2, SC], F8, tag=f"pT{k2}", name=f"pT{k2}",
                                 bufs=2)
                        for k2 in range(ST // 2)]
                  for kt in range(ST):
                      issue_scores(c, kt, pT)
                      # fill PE gaps with the previous chunk's output matmuls
                      if prev is not None and kt % 3 == 2:
                          issue_out(prev[0], (kt - 2) // 3, prev[1])
                  if prev is not None:
                      issue_out(prev[0], 5, prev[1])
                      issue_den(prev[0], prev[1])
                  prev = (c, pT)
              issue_den(prev[0], prev[1])
              for ot in range(DT):
                  issue_out(prev[0], ot, prev[1])

          # ====== Phase C: y = gelu((outT.T @ wo) * inv_den + bo' + x) ======
          # two token tiles per iteration: one PSUM tile [P,2,1024] (1024 so
          # each token is 2-bank aligned), wide DVE/Pool/ACT ops, one out DMA
          if "C" in phases:
           with tc.tile_pool(name="fin", bufs=2) as fin, \
               tc.tile_pool(name="py", bufs=2, space="PSUM") as pyp:
              for t2 in range(ST // 2):
                  ps_y = pyp.tile([P, 2, 1024], F32, tag="ps_y", name="ps_y")
                  for i in range(2):
                      t = 2 * t2 + i
                      for h0, hn in ((0, 512), (512, 256)):
                          for op in range(DP):
                              nc.tensor.matmul(
                                  ps_y[:, i, h0:h0 + hn],
                                  lhsT=outT[op][:, :, ts(t, P)],
                                  rhs=wo_t[op][:, :, h0:h0 + hn],
                                  start=(op == 0), stop=(op == DP - 1),
                                  perf_mode=DR)
                  xb = fin.tile([P, 2, D], F32, tag="xb", name="xb")
                  for i in range(2):
                      nc.gpsimd.tensor_tensor(out=xb[:, i, :],
                                              in0=x_res[2 * t2 + i],
                                              in1=bo_t, op=OP.add)
                  t1 = fin.tile([P, 2, D], F32, tag="t1", name="t1")
                  for i in range(2):
                      t = 2 * t2 + i
                      nc.vector.tensor_scalar(out=t1[:, i, :],
                                              in0=ps_y[:, i, 0:D],
                                              scalar1=inv_den[:, t:t + 1],
                                              scalar2=None, op0=OP.mult)
                  y_t = fin.tile([P, 2, D], F32, tag="y_t", name="y_t")
                  nc.gpsimd.tensor_tensor(out=y_t, in0=t1, in1=xb, op=OP.add)
                  g_t = fin.tile([P, 2, D], F32, tag="g_t", name="g_t")
                  nc.scalar.activation(out=g_t, in_=y_t, func=AF.Gelu)
                  # store on the ACT HWDGE queue so the next rep's x loads
                  # (sync queue) aren't stuck behind 6MB of output drain
                  nc.scalar.dma_start(
                      out=out_d[ts(t2, 2 * P), :].rearrange(
                          "(i p) d -> p i d", p=P),
                      in_=g_t)

    nc.compile()
    return nc


_NC_CACHE = None


def _get_nc():
    global _NC_CACHE
    if _NC_CACHE is None:
        _NC_CACHE = build_bass()
    return _NC_CACHE


def _pair(w):
    """[D, N] -> [DP, P, 2, N] pairing consecutive 128-row K-slices."""
    return np.ascontiguousarray(
        w.reshape(DP, 2, P, -1).transpose(0, 2, 1, 3))


def prep_inputs(x, ln_gamma, ln_beta, w_qkv, b_qkv, w_out, b_out):
    """Host-side weight prep; returns per-core in_maps."""
    x = np.asarray(x, np.float32)
    g = np.asarray(ln_gamma, np.float32)
    be = np.asarray(ln_beta, np.float32)
    w_qkv = np.asarray(w_qkv, np.float32)
    b_qkv = np.asarray(b_qkv, np.float32)
    w_out = np.asarray(w_out, np.float32)
    b_out = np.asarray(b_out, np.float32)

    f8 = mybir.dt.np(F8)
    wg = w_qkv * g[:, None]
    bias = be @ w_qkv + b_qkv
    # v bias folded into the output bias: (p@(v+bv))/den @ wo = out@wo + bv@wo
    bo = b_out + bias[2 * D:] @ w_out
    shared = {
        "wqk": _pair(wg[:, :2 * D]).astype(f8),
        "wv": _pair(wg[:, 2 * D:]).astype(f8),
        "wo": _pair(w_out).astype(f8),
        "bqk": np.ascontiguousarray(bias[:2 * D].reshape(2 * DT, P).T),
        "bo": np.ascontiguousarray(np.broadcast_to(bo, (P, D))),
    }
    return [dict(shared, x=np.ascontiguousarray(x[b])) for b in range(B)]


def kernel(**inputs) -> np.ndarray:
    nc = _get_nc()
    in_maps = prep_inputs(**inputs)
    res = run_bass_kernel_spmd(nc, in_maps, core_ids=list(range(B)))
    return np.stack([res.results[b]["out"] for b in range(B)])


# revision 78
# speedup vs baseline: 1.0985x; 1.0985x over previous
"""Trainium2 Bass kernel for nn_Attention_40372692582854.

Single-head attention block: LayerNorm -> QKV -> softmax(QK^T*sc)@V -> out
projection -> gelu(out + x).  Data-parallel over batch: 8 batch elements,
one per NeuronCore.

All matmuls run as fp8 (e4m3) DoubleRow: a dual-row fp8 matmul contracts
K=256 per pass at the same per-instruction cost as a bf16 K=128 matmul
(measured ~220-235ns per 512-col matmul either way), halving the PE
instruction stream.  Paired operands hold two consecutive 128-row K-slices
in a middle dim of 2 (16B-aligned strides per the dual-fp8 ISA rules).
Measured rel err vs the fp32 reference: 1.7e-2 (gate 2e-2); numerics were
validated operand-by-operand against a numpy fp8 model.

Per-core dataflow (S=2048 tokens, D=768 dims):
  - Phase A, per 512-token group: LN stats via bn_stats; x1c=(x-mu)*rstd in
    bf16 (gamma/beta folded host-side), PE-transposed 128x128 pair-wise into
    fp8 x1cT [128,2,512] tiles.  Then in the same group:
      v[s,dv]   = x1cT.T @ wv   (3 dual mm per h-chunk, fp8 out, bias
                                 folded into bo on host: bo' = bo + bv@wo)
      kT/qT[dk,s] = wqk.T @ x1cT + b  (3 dual mm, bias in the fp8 cast,
                                 unscaled: 1/sqrt(D) is applied in the exp)
    x stays resident in SBUF for the phase-C residual.
  - Phase B, attention per 512-query chunk (all fp8 DoubleRow):
      scoresT[k,q] = kT.T @ qT          (3 dual mm)
      pT = exp(sc*scoresT - ln 16)      (ACT, fp8 out)
      den[1,q] via ones dual mm; outT[dv,q]/16 = v.T @ pT -> fp8 pairs.
    Chunk c-1's output matmuls are interleaved into chunk c's score stream
    so PE never waits on ACT's exp and keeps its p-state high.
  - Phase C: y = gelu((outT.T @ wo) * (16/den) + bo' + x), two token tiles
    per iteration (wide DVE/Pool/ACT ops, single in/out DMAs).
"""

import math
import numpy as np
import ml_dtypes
from contextlib import ExitStack

import concourse.bass as bass
import concourse.tile as tile
import concourse.mybir as mybir
from concourse import bacc
from concourse.masks import make_identity
from concourse.bass_utils import run_bass_kernel_spmd

F32 = mybir.dt.float32
BF16 = mybir.dt.bfloat16
F8 = mybir.dt.float8e4
AF = mybir.ActivationFunctionType
OP = mybir.AluOpType
DR = mybir.MatmulPerfMode.DoubleRow

B = 8
S = 2048
D = 768
P = 128
DT = D // P            # 6 dim tiles
DP = DT // 2           # 3 dim pairs
ST = S // P            # 16 token tiles
SC = 512               # matmul moving free dim
NSC = S // SC          # 4 token chunks
EPS = 1e-5
SCALE = D ** -0.5
PDIV = 16.0            # p = exp(sc*s)/PDIV so fp8 sees well-scaled values


def ts(i, n):
    return bass.ts(i, n)


def build_bass(reps=1, phases="ABC"):
    if "A" in phases:
        phases = phases.replace("A", "LVK")
    nc = bacc.Bacc("TRN2")

    x_d = nc.dram_tensor("x", [S, D], F32, kind="ExternalInput")
    wqk_d = nc.dram_tensor("wqk", [DP, P, 2, 2 * D], F8, kind="ExternalInput")
    wv_d = nc.dram_tensor("wv", [DP, P, 2, D], F8, kind="ExternalInput")
    wo_d = nc.dram_tensor("wo", [DP, P, 2, D], F8, kind="ExternalInput")
    bqk_d = nc.dram_tensor("bqk", [P, 2 * DT], F32, kind="ExternalInput")
    bo_d = nc.dram_tensor("bo", [P, D], F32, kind="ExternalInput")
    out_d = nc.dram_tensor("out", [S, D], F32, kind="ExternalOutput")

    with tile.TileContext(nc) as tc, \
         nc.allow_low_precision(reason="fp8 kernel, validated vs reference"):
      for _rep in range(reps):
        with ExitStack() as ctx:
          const = ctx.enter_context(tc.tile_pool(name="const", bufs=1))
          big = ctx.enter_context(tc.tile_pool(name="big", bufs=1))

          # ---- long-lived constants ----
          wo_t = [const.tile([P, 2, D], F8, tag=f"wo{i}", name=f"wo{i}")
                  for i in range(DP)]
          bo_t = const.tile([P, D], F32, tag="bo", name="bo")
          # [P, 2, 16] so the dual-fp8 ldweights outer free stride is 16B
          ones_t = const.tile([P, 2, 16], F8, tag="ones", name="ones")
          nc.vector.memset(ones_t, 1.0)
          ident = const.tile([P, P], BF16, tag="ident", name="ident")
          make_identity(nc, ident)
          nexp_b = const.tile([P, 1], F32, tag="nexp_b", name="nexp_b")
          nc.vector.memset(nexp_b, -math.log(PDIV))

          # ---- persistent activations (fp8 pairs for DoubleRow) ----
          v_t = [big.tile([P, 2, D], F8, tag=f"v{t2}", name=f"v{t2}")
                 for t2 in range(ST // 2)]
          kT = [big.tile([P, 2, S], F8, tag=f"kT{j}", name=f"kT{j}")
                for j in range(DP)]
          qT = [big.tile([P, 2, S], F8, tag=f"qT{j}", name=f"qT{j}")
                for j in range(DP)]
          inv_den = big.tile([P, ST], F32, tag="inv_den", name="inv_den")
          outT = [big.tile([P, 2, S], F8, tag=f"outT{op}", name=f"outT{op}")
                  for op in range(DP)]
          mvall = big.tile([P, 2 * ST], F32, tag="mvall", name="mvall")
          invall = big.tile([P, ST], F32, tag="invall", name="invall")
          # x stays resident for the phase-C residual (saves a 6MB reload)
          x_res = [big.tile([P, D], F32, tag=f"x{t}", name=f"x{t}")
                   for t in range(ST)]

          # ========= Phase A: LN + transpose + V/K/Q, per token group =======
          if "L" in phases:
           with tc.tile_pool(name="wpool", bufs=1) as wp, \
               tc.tile_pool(name="ln", bufs=6) as ln, \
               tc.tile_pool(name="proj", bufs=2, space="PSUM") as proj, \
               tc.tile_pool(name="xt", bufs=2) as xtp:
              wqk_t = [wp.tile([P, 2, 2 * D], F8, tag=f"wqk{i}", name=f"wqk{i}")
                       for i in range(DP)]
              wv_t = [wp.tile([P, 2, D], F8, tag=f"wv{i}", name=f"wv{i}")
                      for i in range(DP)]
              bqk_t = wp.tile([P, 2 * DT], F32, tag="bqk", name="bqk")
              # weights go on the gpsimd SWDGE queue so the x loads (sync
              # HWDGE) aren't queued behind the weight traffic
              for i in range(DP):
                  nc.gpsimd.dma_start(out=wv_t[i], in_=wv_d[i])
              for i in range(DP):
                  nc.gpsimd.dma_start(out=wqk_t[i], in_=wqk_d[i])
              nc.gpsimd.dma_start(out=bqk_t, in_=bqk_d[:, :])
              for i in range(DP):
                  nc.gpsimd.dma_start(out=wo_t[i], in_=wo_d[i])
              nc.gpsimd.dma_start(out=bo_t, in_=bo_d[:, :])
              eps_t = wp.tile([P, 1], F32, tag="eps", name="eps")
              nc.vector.memset(eps_t, EPS)

              for c in range(NSC):
                  # one tile [P, 2(pair), 3(jp), SC]: dual-pair dim stride
                  # 3*SC=1536B stays 16B-aligned for the fp8 ldweights
                  x1cT = xtp.tile([P, 2, DP, SC], F8, tag="x1cT",
                                  name="x1cT", bufs=2)
                  xts = []
                  for t in range(4 * c, 4 * c + 4):
                      x_t = x_res[t]
                      xts.append(x_t)
                      nc.sync.dma_start(out=x_t, in_=x_d[ts(t, P), :])
                      stats = ln.tile([P, 3, 6], F32, tag="stats", name="stats")
                      for sg in range(3):
                          nc.vector.bn_stats(out=stats[:, sg, :],
                                             in_=x_t[:, ts(sg, 256)])
                      nc.vector.bn_aggr(out=mvall[:, 2 * t:2 * t + 2], in_=stats)
                  # one batched sqrt over the 4 variances (strided AP)
                  stdb = ln.tile([P, 4], F32, tag="stdb", name="stdb")
                  nc.scalar.activation(
                      out=stdb,
                      in_=mvall[:, 8 * c: 8 * c + 8].rearrange(
                          "p (t two) -> p t two", two=2)[:, :, 1],
                      func=AF.Sqrt, bias=eps_t, scale=1.0)
                  nc.vector.reciprocal(out=invall[:, 4 * c:4 * c + 4], in_=stdb)
                  for tt, t in enumerate(range(4 * c, 4 * c + 4)):
                      x1c = ln.tile([P, D], BF16, tag="x1c", name="x1c", bufs=8)
                      nc.vector.tensor_scalar(out=x1c, in0=xts[tt],
                                              scalar1=mvall[:, 2 * t:2 * t + 1],
                                              scalar2=invall[:, t:t + 1],
                                              op0=OP.subtract, op1=OP.mult)
                      # all 6 transposes of the tile into one PSUM tile,
                      # blocks ordered (pair i, jp) so a single copy's AP
                      # iteration matches the x1cT [i, jp, col] layout
                      pst = proj.tile([P, DT * P], BF16, tag="ptr",
                                      name="pst", bufs=3)
                      for i in range(2):
                          for jp in range(DP):
                              nc.tensor.transpose(
                                  pst[:, ts(i * DP + jp, P)],
                                  x1c[:, ts(2 * jp + i, P)], ident)
                      if tt % 2 == 0:
                          nc.vector.tensor_copy(
                              out=x1cT[:, :, :, ts(tt, P)], in_=pst)
                      else:
                          nc.scalar.copy(
                              out=x1cT[:, :, :, ts(tt, P)], in_=pst)

                  # ---- V = x1 @ Wv for these 4 tiles (bias folded in bo) ----
                  for tt, t in enumerate(range(4 * c, 4 * c + 4) if "V" in phases else ()):
                      ps = proj.tile([P, D], F32, tag="mm", name="pv")
                      for h0, hn in ((0, 512), (512, 256)):
                          for jp in range(DP):
                              nc.tensor.matmul(
                                  ps[:, h0:h0 + hn],
                                  lhsT=x1cT[:, :, jp, ts(tt, P)],
                                  rhs=wv_t[jp][:, :, h0:h0 + hn],
                                  start=(jp == 0), stop=(jp == DP - 1),
                                  perf_mode=DR)
                      nc.scalar.copy(out=v_t[t // 2][:, t % 2, :], in_=ps)

                  # ---- kT, qT = W.T @ x1cT + bias for this token chunk ----
                  for which, dst in (((1, kT), (0, qT)) if "K" in phases else ()):
                      for j in range(DT):
                          bcol = bqk_t[:, which * DT + j: which * DT + j + 1]
                          pss = proj.tile([P, SC], F32, tag="mm", name="pkq",
                                          padded_shape=[P, D])
                          for jp in range(DP):
                              nc.tensor.matmul(
                                  pss,
                                  lhsT=wqk_t[jp][:, :, which * D + j * P:
                                                 which * D + (j + 1) * P],
                                  rhs=x1cT[:, :, jp, :],
                                  start=(jp == 0), stop=(jp == DP - 1),
                                  perf_mode=DR)
                          nc.scalar.activation(
                              out=dst[j // 2][:, j % 2, ts(c, SC)], in_=pss,
                              func=AF.Identity, bias=bcol, scale=1.0)

          # ====== Phase B: attention per q-chunk, fp8 DoubleRow, pipelined ==
          if "B" in phases:
           with tc.tile_pool(name="att", bufs=2) as att, \
               tc.tile_pool(name="att2", bufs=2) as att2, \
               tc.tile_pool(name="dram", bufs=2, space="DRAM") as dram, \
               tc.tile_pool(name="ps_s", bufs=4, space="PSUM") as pssp, \
               tc.tile_pool(name="ps_o", bufs=2, space="PSUM") as posp, \
               tc.tile_pool(name="pden", bufs=2, space="PSUM") as pdenp:

              def issue_scores(c, kt, pT):
                  ps_s = pssp.tile([P, SC], F32, tag="ps_s", name="ps_s",
                                   bufs=4)
                  for jj in range(DP):
                      nc.tensor.matmul(ps_s,
                                       lhsT=kT[jj][:, :, ts(kt, P)],
                                       rhs=qT[jj][:, :, ts(c, SC)],
                                       start=(jj == 0), stop=(jj == DP - 1),
                                       perf_mode=DR)
                  # pT = exp(scale*s - ln PDIV), cast to fp8
                  nc.scalar.activation(out=pT[kt // 2][:, kt % 2, :], in_=ps_s,
                                       func=AF.Exp, bias=nexp_b, scale=SCALE)

              def issue_out(c, ot, pT):
                  ps_o = posp.tile([P, SC], F32, tag="ps_o", name="ps_o",
                                   bufs=2)
                  for k2 in range(ST // 2):
                      nc.tensor.matmul(ps_o,
                                       lhsT=v_t[k2][:, :, ts(ot, P)],
                                       rhs=pT[k2],
                                       start=(k2 == 0), stop=(k2 == ST // 2 - 1),
                                       perf_mode=DR)
                  nc.vector.tensor_copy(out=outT[ot // 2][:, ot % 2, ts(c, SC)],
                                        in_=ps_o)

              def issue_den(c, pT):
                  ps_den = pdenp.tile([1, SC], F32, tag="pden", name="ps_den",
                                      bufs=2)
                  for k2 in range(ST // 2):
                      nc.tensor.matmul(ps_den, lhsT=ones_t[:, :, 0:1],
                                       rhs=pT[k2],
                                       start=(k2 == 0), stop=(k2 == ST // 2 - 1),
                                       perf_mode=DR)
                  # denominator -> per-partition reciprocal via DRAM bounce
                  den_row = att2.tile([1, SC], F32, tag="den_row",
                                      name="den_row")
                  nc.vector.tensor_copy(out=den_row, in_=ps_den)
                  den_b = dram.tile([1, SC], F32, tag="den_b", name="den_b")
                  nc.sync.dma_start(out=den_b, in_=den_row)
                  den_pp = att2.tile([P, NSC], F32, tag="den_pp", name="den_pp")
                  nc.sync.dma_start(out=den_pp,
                                    in_=den_b.rearrange("a (t p) -> (a p) t",
                                                        p=P))
                  nc.vector.reciprocal(out=inv_den[:, c * NSC:(c + 1) * NSC],
                                       in_=den_pp)

              prev = None
              for c in range(NSC):
                  pT = [att.tile([P, 2, SC], F8, tag=f"pT{k2}", name=f"pT{k2}",
                                 bufs=2)
                        for k2 in range(ST // 2)]
                  for kt in range(ST):
                      issue_scores(c, kt, pT)
                      # fill PE gaps with the previous chunk's output matmuls
                      if prev is not None and kt % 3 == 2:
                          issue_out(prev[0], (kt - 2) // 3, prev[1])
                  if prev is not None:
                      issue_out(prev[0], 5, prev[1])
                      issue_den(prev[0], prev[1])
                  prev = (c, pT)
              issue_den(prev[0], prev[1])
              for ot in range(DT):
                  issue_out(prev[0], ot, prev[1])

          # ====== Phase C: y = gelu((outT.T @ wo) * inv_den + bo' + x) ======
          # two token tiles per iteration: one PSUM tile [P,2,1024] (1024 so
          # each token is 2-bank aligned), wide DVE/Pool/ACT ops, one out DMA
          if "C" in phases:
           with tc.tile_pool(name="fin", bufs=2) as fin, \
               tc.tile_pool(name="py", bufs=2, space="PSUM") as pyp:
              for t2 in range(ST // 2):
                  ps_y = pyp.tile([P, 2, 1024], F32, tag="ps_y", name="ps_y")
                  for i in range(2):
                      t = 2 * t2 + i
                      for h0, hn in ((0, 512), (512, 256)):
                          for op in range(DP):
                              nc.tensor.matmul(
                                  ps_y[:, i, h0:h0 + hn],
                                  lhsT=outT[op][:, :, ts(t, P)],
                                  rhs=wo_t[op][:, :, h0:h0 + hn],
                                  start=(op == 0), stop=(op == DP - 1),
                                  perf_mode=DR)
                  xb = fin.tile([P, 2, D], F32, tag="xb", name="xb")
                  for i in range(2):
                      nc.gpsimd.tensor_tensor(out=xb[:, i, :],
                                              in0=x_res[2 * t2 + i],
                                              in1=bo_t, op=OP.add)
                  t1 = fin.tile([P, 2, D], F32, tag="t1", name="t1")
                  for i in range(2):
                      t = 2 * t2 + i
                      nc.vector.tensor_scalar(out=t1[:, i, :],
                                              in0=ps_y[:, i, 0:D],
                                              scalar1=inv_den[:, t:t + 1],
                                              scalar2=None, op0=OP.mult)
                  y_t = fin.tile([P, 2, D], F32, tag="y_t", name="y_t")
                  nc.gpsimd.tensor_tensor(out=y_t, in0=t1, in1=xb, op=OP.add)
                  g_t = fin.tile([P, 2, D], F32, tag="g_t", name="g_t")
                  nc.scalar.activation(out=g_t, in_=y_t, func=AF.Gelu)
                  # store on the ACT HWDGE queue so the next rep's x loads
                  # (sync queue) aren't stuck behind 6MB of output drain
                  nc.scalar.dma_start(
                      out=out_d[ts(t2, 2 * P), :].rearrange(
                          "(i p) d -> p i d", p=P),
                      in_=g_t)

    nc.compile()
    return nc


_NC_CACHE = None


def _get_nc():
    global _NC_CACHE
    if _NC_CACHE is None:
        _NC_CACHE = build_bass()
    return _NC_CACHE


def _pair(w):
    """[D, N] -> [DP, P, 2, N] pairing consecutive 128-row K-slices."""
    return np.ascontiguousarray(
        w.reshape(DP, 2, P, -1).transpose(0, 2, 1, 3))


def prep_inputs(x, ln_gamma, ln_beta, w_qkv, b_qkv, w_out, b_out):
    """Host-side weight prep; returns per-core in_maps."""
    x = np.asarray(x, np.float32)
    g = np.asarray(ln_gamma, np.float32)
    be = np.asarray(ln_beta, np.float32)
    w_qkv = np.asarray(w_qkv, np.float32)
    b_qkv = np.asarray(b_qkv, np.float32)
    w_out = np.asarray(w_out, np.float32)
    b_out = np.asarray(b_out, np.float32)

    f8 = mybir.dt.np(F8)
    wg = w_qkv * g[:, None]
    bias = be @ w_qkv + b_qkv
    # v bias folded into the output bias: (p@(v+bv))/den @ wo = out@wo + bv@wo
    bo = b_out + bias[2 * D:] @ w_out
    shared = {
        "wqk": _pair(wg[:, :2 * D]).astype(f8),
        "wv": _pair(wg[:, 2 * D:]).astype(f8),
        "wo": _pair(w_out).astype(f8),
        "bqk": np.ascontiguousarray(bias[:2 * D].reshape(2 * DT, P).T),
        "bo": np.ascontiguousarray(np.broadcast_to(bo, (P, D))),
    }
    return [dict(shared, x=np.ascontiguousarray(x[b])) for b in range(B)]


def kernel(**inputs) -> np.ndarray:
    nc = _get_nc()
    in_maps = prep_inputs(**inputs)
    res = run_bass_kernel_spmd(nc, in_maps, core_ids=list(range(B)))
    return np.stack([res.results[b]["out"] for b in range(B)])
